# revision 1
# baseline (speedup 1.0000x reference)
"""GCN (3x GCNConv + readout) on 8 Trainium2 NeuronCores.

Strategy (graph/data parallel over destination nodes):
  - Node rows are sharded across 8 cores by destination; each core owns its
    node shard and all edges pointing into it. Weights are replicated.
  - Math reformulation: with a = deg^-0.5 and x' = a*x (prescaled rows),
        layer(x) = relu(a*( (A0 @ x' + x'_self) @ W ) + b)
    where A0 is the *unweighted* 0/1 adjacency. The per-edge norm
    a[src]*a[dst] factorizes away entirely.
  - Per 128-row destination block, the aggregation A0 @ x' is computed as a
    sum of PE matmuls accumulated in PSUM: for each 128-edge tile,
        psum[d, r] += msg_tile[e, d]^T @ sel_tile[e, r]
    where msg_tile rows are gathered source rows (dma_gather, 512 B/row) and
    sel_tile[e, r] = (dst_local[e] == r) is built on DVE via is_equal against
    an IOTA constant. The self term is one extra matmul against identity.
    Output rows are produced in order -> plain sequential DMA writes.
  - dma_gather indices are int16; the node table is addressed through 4
    fixed 32768-row windows with window-relative indices.
  - Between layers, 6.4 MB node shards are AllGather'd so every core has the
    full prescaled table for the next layer's gathers.
"""

import math
from contextlib import ExitStack
from dataclasses import dataclass, field

import numpy as np

P = 128
NW = 4            # gather windows
WSIZE = 32768     # int16 window
CH = 6            # blocks per chunk (gather granularity)
D = 128           # feature dim
O = 16            # readout dim
N_CORES = 8


@dataclass
class Schedule:
    n: int                     # real node count
    ns0: int                   # real nodes per core
    nsp: int                   # padded nodes per core (mult of 128)
    npad: int                  # padded total nodes
    nblocks: int               # blocks per core
    chunks: list               # list of block-lists
    tw: np.ndarray             # [nblocks, NW] tiles per (block, window)
    tile_pos: np.ndarray       # [nblocks, NW] chunk-local tile offset
    call_tile_off: np.ndarray  # [nchunks, NW]
    call_icol_off: np.ndarray  # [nchunks, NW]
    tiles_cw: np.ndarray       # [nchunks, NW]
    chunk_tiles: np.ndarray    # [nchunks]
    chunk_icols: np.ndarray    # [nchunks]
    chunk_tile_base: np.ndarray
    chunk_icol_base: np.ndarray
    total_tiles: int
    total_icols: int
    # per-core data
    idx_arrs: list = field(default_factory=list)   # [128, total_icols] int16
    dl_arrs: list = field(default_factory=list)    # [128, total_tiles] f32
    a_packed: list = field(default_factory=list)   # [128, nblocks] f32
    a_pad: np.ndarray | None = None                # [npad] f32


def build_schedule(edge_index: np.ndarray, n: int, ns0: int) -> Schedule:
    src, dst = edge_index[0].astype(np.int64), edge_index[1].astype(np.int64)
    e = src.shape[0]
    nsp = ((ns0 + P - 1) // P) * P
    npad = N_CORES * nsp
    nblocks = nsp // P
    assert npad <= NW * WSIZE

    deg = (1.0 + np.bincount(dst, minlength=n)).astype(np.float32)
    a = deg ** np.float32(-0.5)
    a_pad = np.ones(npad, np.float32)
    nodes = np.arange(n, dtype=np.int64)
    pid_map = (nodes // ns0) * nsp + nodes % ns0
    a_pad[pid_map] = a

    src_pid = (src // ns0) * nsp + src % ns0
    k_arr = dst // ns0
    dst_loc = dst % ns0
    b_arr = dst_loc // P
    dl_arr = (dst_loc % P).astype(np.float32)
    w_arr = src_pid // WSIZE
    idx16 = (src_pid - w_arr * WSIZE).astype(np.int16)

    ngroups = N_CORES * nblocks * NW
    key = (k_arr * nblocks + b_arr) * NW + w_arr
    cnt = np.bincount(key, minlength=ngroups).reshape(N_CORES, nblocks, NW)
    tw = np.ceil(cnt.max(axis=0) / P).astype(np.int64)          # [nblocks, NW]

    chunks = [list(range(c, min(c + CH, nblocks))) for c in range(0, nblocks, CH)]
    nchunks = len(chunks)

    tile_pos = np.zeros((nblocks, NW), np.int64)
    call_tile_off = np.zeros((nchunks, NW), np.int64)
    call_icol_off = np.zeros((nchunks, NW), np.int64)
    tiles_cw = np.zeros((nchunks, NW), np.int64)
    chunk_tiles = np.zeros(nchunks, np.int64)
    chunk_icols = np.zeros(nchunks, np.int64)
    for c, bl in enumerate(chunks):
        off = 0
        ioff = 0
        for w in range(NW):
            call_tile_off[c, w] = off
            call_icol_off[c, w] = ioff
            for b in bl:
                tile_pos[b, w] = off
                off += tw[b, w]
            tiles_cw[c, w] = off - call_tile_off[c, w]
            ioff += tiles_cw[c, w] * 8          # 128 slots/tile -> 8 int16 cols
        chunk_tiles[c] = off
        chunk_icols[c] = ioff
    chunk_tile_base = np.concatenate([[0], np.cumsum(chunk_tiles)[:-1]])
    chunk_icol_base = np.concatenate([[0], np.cumsum(chunk_icols)[:-1]])
    total_tiles = int(chunk_tiles.sum())
    total_icols = int(chunk_icols.sum())

    sched = Schedule(
        n=n, ns0=ns0, nsp=nsp, npad=npad, nblocks=nblocks, chunks=chunks,
        tw=tw, tile_pos=tile_pos, call_tile_off=call_tile_off,
        call_icol_off=call_icol_off, tiles_cw=tiles_cw,
        chunk_tiles=chunk_tiles, chunk_icols=chunk_icols,
        chunk_tile_base=chunk_tile_base, chunk_icol_base=chunk_icol_base,
        total_tiles=total_tiles, total_icols=total_icols, a_pad=a_pad,
    )

    # per-edge slot assignment (vectorized), per core
    order = np.argsort(key, kind="stable")
    grp_start = np.zeros(ngroups + 1, np.int64)
    np.cumsum(cnt.reshape(-1), out=grp_start[1:])
    rank = np.arange(e, dtype=np.int64) - grp_start[key[order]]

    ch_of_b = np.array([b // CH for b in range(nblocks)], np.int64)
    total_slots = total_tiles * P
    for k in range(N_CORES):
        sel = k_arr[order] == k
        eo = order[sel]
        r = rank[sel]
        b = b_arr[eo]
        w = w_arr[eo]
        c = ch_of_b[b]
        stl = tile_pos[b, w]                      # chunk-local seg tile off
        part = r % P
        ctl = stl + r // P                        # chunk-local tile
        gtl = sched.chunk_tile_base[c] + ctl      # global tile
        # dl array
        dl_core = np.full((P, total_tiles), -1.0, np.float32)
        dl_core[part, gtl] = dl_arr[eo]
        # slot values in global slot order (tile*128 + partition)
        slot_vals = np.full(total_slots, -1, np.int32)
        slot_vals[gtl * P + part] = idx16[eo].astype(np.int32)
        # Pad slots must hold valid in-window indices (interior -1 is
        # illegal), but a constant pad value creates an HBM hotspot that
        # slows the whole gather ~1.7x. Cycle each call's own real indices
        # into its pad slots instead.
        idx_core = np.zeros((P, total_icols), np.int16)
        for c2 in range(len(chunks)):
            for w2 in range(NW):
                ntw = int(tiles_cw[c2, w2])
                if ntw == 0:
                    continue
                s0 = (sched.chunk_tile_base[c2] + call_tile_off[c2, w2]) * P
                vals = slot_vals[s0 : s0 + ntw * P].copy()
                pad = vals < 0
                real = vals[~pad]
                npd = int(pad.sum())
                if npd:
                    if len(real):
                        vals[pad] = real[np.arange(npd) % len(real)]
                    else:
                        vals[pad] = 0
                jj = np.arange(ntw * P)
                ic = sched.chunk_icol_base[c2] + call_icol_off[c2, w2] + jj // 16
                rows = (jj % 16)[None, :] + 16 * np.arange(8)[:, None]
                idx_core[rows, ic[None, :]] = vals.astype(np.int16)[None, :]
        sched.idx_arrs.append(idx_core)
        sched.dl_arrs.append(dl_core)
        ap = np.empty((P, nblocks), np.float32)
        ap[:] = a_pad[k * nsp : (k + 1) * nsp].reshape(nblocks, P).T
        sched.a_packed.append(ap)
    return sched


def build_nc(s: Schedule, repeats: int = 1, skip: frozenset = frozenset()):
    import concourse.bacc as bacc
    import concourse.mybir as mybir
    import concourse.tile as tile
    from concourse import library_config

    f32 = mybir.dt.float32
    i16 = mybir.dt.int16
    AF = mybir.ActivationFunctionType
    OP = mybir.AluOpType

    nc = bacc.Bacc("TRN2", target_bir_lowering=False, debug=False,
                   num_devices=N_CORES)

    x1 = nc.dram_tensor("x1", [s.npad, D], f32, kind="ExternalInput")
    xown0 = nc.dram_tensor("xown0", [s.nsp, D], f32, kind="ExternalInput")
    idx_all = nc.dram_tensor("idx_all", [P, s.total_icols], i16,
                             kind="ExternalInput")
    dl_all = nc.dram_tensor("dl_all", [P, s.total_tiles], f32,
                            kind="ExternalInput")
    a_pk = nc.dram_tensor("a_pk", [P, s.nblocks], f32, kind="ExternalInput")
    w_in = [nc.dram_tensor(f"w{i}", [D, D], f32, kind="ExternalInput")
            for i in range(3)]
    brep_in = [nc.dram_tensor(f"brep{i}", [P, D], f32, kind="ExternalInput")
               for i in range(3)]
    wr_in = nc.dram_tensor("wr", [D, O], f32, kind="ExternalInput")
    brr_in = nc.dram_tensor("brr", [P, O], f32, kind="ExternalInput")
    iota_in = nc.dram_tensor("iota", [P, P], f32, kind="ExternalInput")
    ident_in = nc.dram_tensor("ident", [P, P], f32, kind="ExternalInput")
    out = nc.dram_tensor("out", [s.nsp, O], f32, kind="ExternalOutput")

    shard = [nc.dram_tensor(f"shard{i}", [s.nsp, D], f32, kind="Internal")
             for i in range(2)]
    xfull = [nc.dram_tensor(f"xfull{i}", [s.npad, D], f32, kind="Internal",
                            addr_space="Shared") for i in range(2)]

    max_tiles = int(s.chunk_tiles.max())
    max_icols = int(s.chunk_icols.max())

    with tile.TileContext(nc) as tc, ExitStack() as ctx:
        nc.gpsimd.load_library(library_config.mlp)
        cp = ctx.enter_context(tc.tile_pool(name="consts", bufs=1))
        msgp = ctx.enter_context(tc.tile_pool(name="msg", bufs=2))
        idxp = ctx.enter_context(tc.tile_pool(name="idx", bufs=2))
        dlp = ctx.enter_context(tc.tile_pool(name="dl", bufs=2))
        selp = ctx.enter_context(tc.tile_pool(name="sel", bufs=4))
        xop = ctx.enter_context(tc.tile_pool(name="xo", bufs=3))
        gp = ctx.enter_context(tc.tile_pool(name="g", bufs=3))
        vp = ctx.enter_context(tc.tile_pool(name="v", bufs=3))
        smp = ctx.enter_context(tc.tile_pool(name="sm", bufs=3))
        pgp = ctx.enter_context(tc.tile_pool(name="pg", bufs=2, space="PSUM"))
        p2p = ctx.enter_context(tc.tile_pool(name="p2", bufs=2, space="PSUM"))
        p3p = ctx.enter_context(tc.tile_pool(name="p3", bufs=2, space="PSUM"))
        p4p = ctx.enter_context(tc.tile_pool(name="p4", bufs=2, space="PSUM"))

        w_t, brep_t = [], []
        for i in range(3):
            t = cp.tile([D, D], f32, tag=f"w{i}")
            nc.sync.dma_start(out=t[:], in_=w_in[i].ap()[:])
            w_t.append(t)
            t = cp.tile([P, D], f32, tag=f"brep{i}")
            nc.sync.dma_start(out=t[:], in_=brep_in[i].ap()[:])
            brep_t.append(t)
        wr_t = cp.tile([D, O], f32, tag="wr")
        nc.sync.dma_start(out=wr_t[:], in_=wr_in.ap()[:])
        brr_t = cp.tile([P, O], f32, tag="brr")
        nc.sync.dma_start(out=brr_t[:], in_=brr_in.ap()[:])
        iota_t = cp.tile([P, P], f32, tag="iota")
        nc.sync.dma_start(out=iota_t[:], in_=iota_in.ap()[:])
        ident_t = cp.tile([P, P], f32, tag="ident")
        nc.sync.dma_start(out=ident_t[:], in_=ident_in.ap()[:])
        apk_t = cp.tile([P, s.nblocks], f32, tag="apk")
        nc.sync.dma_start(out=apk_t[:], in_=a_pk.ap()[:])

        if "localtab" in skip:
            tables = [x1, x1, x1]
        else:
            tables = [x1, xfull[0], xfull[1]]
        xowns = [xown0, shard[0], shard[1]]

        for _rep in range(repeats):
            _build_pass(nc, tc, s, tables, xowns, shard, xfull, out,
                        w_t, brep_t, wr_t, brr_t, iota_t, apk_t, ident_t,
                        msgp, idxp, dlp, selp, xop, gp, vp, smp,
                        pgp, p2p, p3p, p4p,
                        idx_all, dl_all, max_tiles, max_icols, skip)
    nc.compile()
    return nc


def _build_pass(nc, tc, s, tables, xowns, shard, xfull, out,
                w_t, brep_t, wr_t, brr_t, iota_t, apk_t, ident_t,
                msgp, idxp, dlp, selp, xop, gp, vp, smp,
                pgp, p2p, p3p, p4p,
                idx_all, dl_all, max_tiles, max_icols, skip=frozenset()):
    import concourse.mybir as mybir

    f32 = mybir.dt.float32
    i16 = mybir.dt.int16
    AF = mybir.ActivationFunctionType
    OP = mybir.AluOpType
    if True:
        for layer in range(3):
            table_ap = tables[layer].ap()
            xown_ap = xowns[layer].ap()
            for c, bl in enumerate(s.chunks):
                tiles_c = int(s.chunk_tiles[c])
                icols_c = int(s.chunk_icols[c])
                msg_t = msgp.tile([P, max_tiles, D], f32, tag="msg")
                idx_t = idxp.tile([P, max_icols], i16, tag="idx")
                dl_t = dlp.tile([P, max_tiles], f32, tag="dl")
                ic0 = int(s.chunk_icol_base[c])
                tb0 = int(s.chunk_tile_base[c])
                nc.sync.dma_start(out=idx_t[:, :icols_c],
                                  in_=idx_all.ap()[:, ic0 : ic0 + icols_c])
                nc.sync.dma_start(out=dl_t[:, :tiles_c],
                                  in_=dl_all.ap()[:, tb0 : tb0 + tiles_c])
                if "gather" in skip:
                    nc.vector.memset(msg_t[:, 0, :], 0.0)
                for w in range(NW):
                    if "gather" in skip:
                        continue
                    ntw = int(s.tiles_cw[c, w])
                    if ntw == 0:
                        continue
                    to = int(s.call_tile_off[c, w])
                    io = int(s.call_icol_off[c, w])
                    assert ntw * P <= 8192, "dma_gather call too large for HW"
                    nc.gpsimd.dma_gather(
                        msg_t[:, to : to + ntw, :],
                        table_ap[WSIZE * w :, :],
                        idx_t[:, io : io + ntw * 8],
                        ntw * P,
                        ntw * P,
                        D,
                        single_packet=False,
                    )
                if "compute" in skip:
                    for b in bl:
                        if layer < 2:
                            nc.sync.dma_start(
                                out=shard[layer].ap()[b * P : (b + 1) * P, :],
                                in_=msg_t[:, int(s.tile_pos[b, 0]), :])
                    continue
                for b in bl:
                    psum_g = pgp.tile([P, P], f32, tag="pg")
                    first = True
                    for w in range(NW):
                        for t in range(int(s.tw[b, w])):
                            ctl = int(s.tile_pos[b, w]) + t
                            if "sel" in skip:
                                sel = iota_t
                            else:
                                sel = selp.tile([P, P], f32, tag="sel")
                                nc.vector.tensor_tensor(
                                    out=sel[:],
                                    in0=dl_t[:, ctl : ctl + 1].to_broadcast([P, P]),
                                    in1=iota_t[:],
                                    op=OP.is_equal,
                                )
                            if "aggmm" not in skip:
                                nc.tensor.matmul(
                                    out=psum_g[:], lhsT=msg_t[:, ctl, :],
                                    rhs=sel[:], start=first, stop=False,
                                )
                                first = False
                    xo = xop.tile([P, D], f32, tag="xo")
                    nc.sync.dma_start(out=xo[:],
                                      in_=xown_ap[b * P : (b + 1) * P, :])
                    nc.tensor.matmul(out=psum_g[:], lhsT=xo[:], rhs=ident_t[:],
                                     start=first, stop=True)
                    g_sb = gp.tile([P, P], f32, tag="g")
                    nc.vector.tensor_copy(out=g_sb[:], in_=psum_g[:])
                    psum2 = p2p.tile([P, D], f32, tag="p2")
                    nc.tensor.matmul(out=psum2[:], lhsT=g_sb[:],
                                     rhs=w_t[layer][:], start=True, stop=True)
                    acol = apk_t[:, b : b + 1]
                    v = vp.tile([P, D], f32, tag="v")
                    nc.vector.tensor_scalar(
                        out=v[:], in0=psum2[:], scalar1=acol, scalar2=None,
                        op0=OP.mult,
                    )
                    wv = vp.tile([P, D], f32, tag="wv")
                    nc.vector.tensor_tensor(out=wv[:], in0=v[:],
                                            in1=brep_t[layer][:], op=OP.add)
                    if layer < 2:
                        xn = smp.tile([P, D], f32, tag="xn")
                        nc.scalar.activation(xn[:], wv[:], AF.Relu, scale=acol)
                        nc.sync.dma_start(
                            out=shard[layer].ap()[b * P : (b + 1) * P, :],
                            in_=xn[:])
                    else:
                        o3 = smp.tile([P, D], f32, tag="o3")
                        nc.scalar.activation(o3[:], wv[:], AF.Relu)
                        psum3 = p3p.tile([P, P], f32, tag="p3")
                        nc.tensor.transpose(out=psum3[:], in_=o3[:],
                                            identity=ident_t[:])
                        tt = gp.tile([P, P], f32, tag="tt")
                        nc.vector.tensor_copy(out=tt[:], in_=psum3[:])
                        psum4 = p4p.tile([P, O], f32, tag="p4")
                        nc.tensor.matmul(out=psum4[:], lhsT=tt[:], rhs=wr_t[:],
                                         start=True, stop=True)
                        zr = smp.tile([P, O], f32, tag="zr")
                        nc.vector.tensor_tensor(out=zr[:], in0=psum4[:],
                                                in1=brr_t[:], op=OP.add)
                        sg = smp.tile([P, O], f32, tag="sg")
                        nc.scalar.activation(sg[:], zr[:], AF.Sigmoid)
                        ro = smp.tile([P, O], f32, tag="ro")
                        nc.vector.tensor_scalar(
                            out=ro[:], in0=sg[:], scalar1=0.8, scalar2=0.1,
                            op0=OP.mult, op1=OP.add,
                        )
                        nc.sync.dma_start(
                            out=out.ap()[b * P : (b + 1) * P, :], in_=ro[:])
            if layer < 2 and "ag" not in skip:
                nc.gpsimd.collective_compute(
                    "AllGather",
                    mybir.AluOpType.bypass,
                    replica_groups=[list(range(N_CORES))],
                    ins=[shard[layer].ap()[:]],
                    outs=[xfull[layer].ap()[:]],
                )


def build_inmaps(s: Schedule, x: np.ndarray, W0, b0, W1, b1, W2, b2, Wr, br):
    x_pad = np.zeros((s.npad, D), np.float32)
    nodes = np.arange(s.n, dtype=np.int64)
    pid_map = (nodes // s.ns0) * s.nsp + nodes % s.ns0
    x_pad[pid_map] = x
    x1 = x_pad * s.a_pad[:, None]

    consts = {
        "x1": x1,
        "w0": np.asarray(W0, np.float32), "w1": np.asarray(W1, np.float32),
        "w2": np.asarray(W2, np.float32),
        "brep0": np.tile(np.asarray(b0, np.float32), (P, 1)),
        "brep1": np.tile(np.asarray(b1, np.float32), (P, 1)),
        "brep2": np.tile(np.asarray(b2, np.float32), (P, 1)),
        "wr": np.asarray(Wr, np.float32),
        "brr": np.tile(np.asarray(br, np.float32), (P, 1)),
        "iota": np.tile(np.arange(P, dtype=np.float32), (P, 1)),
        "ident": np.eye(P, dtype=np.float32),
    }
    in_maps = []
    for k in range(N_CORES):
        m = dict(consts)
        m["xown0"] = np.ascontiguousarray(x1[k * s.nsp : (k + 1) * s.nsp])
        m["idx_all"] = s.idx_arrs[k]
        m["dl_all"] = s.dl_arrs[k]
        m["a_pk"] = s.a_packed[k]
        in_maps.append(m)
    return in_maps


def assemble_output(s: Schedule, results: list) -> np.ndarray:
    out = np.empty((s.n, O), np.float32)
    for k in range(N_CORES):
        lo = k * s.ns0
        hi = min((k + 1) * s.ns0, s.n)
        out[lo:hi] = results[k]["out"][: hi - lo]
    return out


def run(x, edge_index, W0, b0, W1, b1, W2, b2, Wr, br, n, ns0, **run_kwargs):
    from concourse.bass_utils import run_bass_kernel_spmd

    s = build_schedule(np.asarray(edge_index), n, ns0)
    nc = build_nc(s)
    in_maps = build_inmaps(s, np.asarray(x, np.float32), W0, b0, W1, b1, W2,
                           b2, Wr, br)
    res = run_bass_kernel_spmd(nc, in_maps, core_ids=list(range(N_CORES)),
                               **run_kwargs)
    return assemble_output(s, res.results), res


def kernel(x, edge_index, W0, b0, W1, b1, W2, b2, Wr, br):
    out, _ = run(x, edge_index, W0, b0, W1, b1, W2, b2, Wr, br,
                 n=100000, ns0=12500)
    return out



# revision 13
# speedup vs baseline: 1.0774x; 1.0774x over previous
"""GCN (3x GCNConv + readout) on 8 Trainium2 NeuronCores.

Strategy (graph/data parallel over destination nodes):
  - Node rows are sharded across 8 cores by destination; each core owns its
    node shard and all edges pointing into it. Weights are replicated.
  - Math reformulation: with a = deg^-0.5 and x' = a*x (prescaled rows),
        layer(x) = relu(a*( (A0 @ x' + x'_self) @ W ) + b)
    where A0 is the *unweighted* 0/1 adjacency. The per-edge norm
    a[src]*a[dst] factorizes away entirely.
  - Per 128-row destination block, the aggregation A0 @ x' is computed as a
    sum of PE matmuls accumulated in PSUM: for each 128-edge tile,
        psum[d, r] += msg_tile[e, d]^T @ sel_tile[e, r]
    where msg_tile rows are gathered source rows (dma_gather, 512 B/row) and
    sel_tile[e, r] = (dst_local[e] == r) is built on DVE via is_equal against
    an IOTA constant. The self term is one extra matmul against identity.
    Output rows are produced in order -> plain sequential DMA writes.
  - dma_gather indices are int16; the node table is addressed through 4
    fixed 32768-row windows with window-relative indices.
  - Between layers, 6.4 MB node shards are AllGather'd so every core has the
    full prescaled table for the next layer's gathers.
"""

import math
from contextlib import ExitStack
from dataclasses import dataclass, field

import ml_dtypes
import numpy as np

BF16 = ml_dtypes.bfloat16

P = 128
NW = 4            # gather windows
NQUEUES = 1       # SWDGE queues (gather descgen core pairs)
WSIZE = 32768     # int16 window
CH = 6            # blocks per chunk (gather granularity)
D = 128           # feature dim
O = 16            # readout dim
N_CORES = 8


@dataclass
class Schedule:
    n: int                     # real node count
    ns0: int                   # real nodes per core
    nsp: int                   # padded nodes per core (mult of 128)
    npad: int                  # padded total nodes
    nblocks: int               # blocks per core
    chunks: list               # list of block-lists
    tw: np.ndarray             # [nblocks, NW] tiles per (block, window)
    tile_pos: np.ndarray       # [nblocks, NW] chunk-local tile offset
    call_tile_off: np.ndarray  # [nchunks, NW]
    call_icol_off: np.ndarray  # [nchunks, NW]
    tiles_cw: np.ndarray       # [nchunks, NW]
    chunk_tiles: np.ndarray    # [nchunks]
    chunk_icols: np.ndarray    # [nchunks]
    chunk_tile_base: np.ndarray
    chunk_icol_base: np.ndarray
    total_tiles: int
    total_icols: int
    # per-core data
    idx_arrs: list = field(default_factory=list)   # [128, total_icols] int16
    dl_arrs: list = field(default_factory=list)    # [128, total_tiles] f32
    a_packed: list = field(default_factory=list)   # [128, nblocks] f32
    a_pad: np.ndarray | None = None                # [npad] f32


def build_schedule(edge_index: np.ndarray, n: int, ns0: int) -> Schedule:
    src, dst = edge_index[0].astype(np.int64), edge_index[1].astype(np.int64)
    e = src.shape[0]
    nsp = ((ns0 + P - 1) // P) * P
    npad = N_CORES * nsp
    nblocks = nsp // P
    assert npad <= NW * WSIZE

    deg = (1.0 + np.bincount(dst, minlength=n)).astype(np.float32)
    a = deg ** np.float32(-0.5)
    a_pad = np.ones(npad, np.float32)
    nodes = np.arange(n, dtype=np.int64)
    pid_map = (nodes // ns0) * nsp + nodes % ns0
    a_pad[pid_map] = a

    src_pid = (src // ns0) * nsp + src % ns0
    k_arr = dst // ns0
    dst_loc = dst % ns0
    b_arr = dst_loc // P
    dl_arr = (dst_loc % P).astype(np.float32)
    w_arr = src_pid // WSIZE
    idx16 = (src_pid - w_arr * WSIZE).astype(np.int16)

    ngroups = N_CORES * nblocks * NW
    key = (k_arr * nblocks + b_arr) * NW + w_arr
    cnt = np.bincount(key, minlength=ngroups).reshape(N_CORES, nblocks, NW)
    tw = np.ceil(cnt.max(axis=0) / P).astype(np.int64)          # [nblocks, NW]

    chunks = [list(range(c, min(c + CH, nblocks))) for c in range(0, nblocks, CH)]
    nchunks = len(chunks)

    tile_pos = np.zeros((nblocks, NW), np.int64)
    call_tile_off = np.zeros((nchunks, NW), np.int64)
    call_icol_off = np.zeros((nchunks, NW), np.int64)
    tiles_cw = np.zeros((nchunks, NW), np.int64)
    chunk_tiles = np.zeros(nchunks, np.int64)
    chunk_icols = np.zeros(nchunks, np.int64)
    for c, bl in enumerate(chunks):
        off = 0
        ioff = 0
        for w in range(NW):
            call_tile_off[c, w] = off
            call_icol_off[c, w] = ioff
            for b in bl:
                tile_pos[b, w] = off
                off += tw[b, w]
            tiles_cw[c, w] = off - call_tile_off[c, w]
            ioff += tiles_cw[c, w] * 8          # 128 slots/tile -> 8 int16 cols
        chunk_tiles[c] = off
        chunk_icols[c] = ioff
    chunk_tile_base = np.concatenate([[0], np.cumsum(chunk_tiles)[:-1]])
    chunk_icol_base = np.concatenate([[0], np.cumsum(chunk_icols)[:-1]])
    total_tiles = int(chunk_tiles.sum())
    total_icols = int(chunk_icols.sum())

    sched = Schedule(
        n=n, ns0=ns0, nsp=nsp, npad=npad, nblocks=nblocks, chunks=chunks,
        tw=tw, tile_pos=tile_pos, call_tile_off=call_tile_off,
        call_icol_off=call_icol_off, tiles_cw=tiles_cw,
        chunk_tiles=chunk_tiles, chunk_icols=chunk_icols,
        chunk_tile_base=chunk_tile_base, chunk_icol_base=chunk_icol_base,
        total_tiles=total_tiles, total_icols=total_icols, a_pad=a_pad,
    )

    # per-edge slot assignment (vectorized), per core
    order = np.argsort(key, kind="stable")
    grp_start = np.zeros(ngroups + 1, np.int64)
    np.cumsum(cnt.reshape(-1), out=grp_start[1:])
    rank = np.arange(e, dtype=np.int64) - grp_start[key[order]]

    ch_of_b = np.array([b // CH for b in range(nblocks)], np.int64)
    total_slots = total_tiles * P
    for k in range(N_CORES):
        sel = k_arr[order] == k
        eo = order[sel]
        r = rank[sel]
        b = b_arr[eo]
        w = w_arr[eo]
        c = ch_of_b[b]
        stl = tile_pos[b, w]                      # chunk-local seg tile off
        part = r % P
        ctl = stl + r // P                        # chunk-local tile
        gtl = sched.chunk_tile_base[c] + ctl      # global tile
        # dl array
        dl_core = np.full((P, total_tiles), -1.0, np.float32)
        dl_core[part, gtl] = dl_arr[eo]
        # slot values in global slot order (tile*128 + partition)
        slot_vals = np.full(total_slots, -1, np.int32)
        slot_vals[gtl * P + part] = idx16[eo].astype(np.int32)
        # Pad slots must hold valid in-window indices (interior -1 is
        # illegal), but a constant pad value creates an HBM hotspot that
        # slows the whole gather ~1.7x. Cycle each call's own real indices
        # into its pad slots instead.
        idx_core = np.zeros((P, total_icols), np.int16)
        for c2 in range(len(chunks)):
            for w2 in range(NW):
                ntw = int(tiles_cw[c2, w2])
                if ntw == 0:
                    continue
                s0 = (sched.chunk_tile_base[c2] + call_tile_off[c2, w2]) * P
                vals = slot_vals[s0 : s0 + ntw * P].copy()
                pad = vals < 0
                real = vals[~pad]
                npd = int(pad.sum())
                if npd:
                    if len(real):
                        vals[pad] = real[np.arange(npd) % len(real)]
                    else:
                        vals[pad] = 0
                jj = np.arange(ntw * P)
                ic = sched.chunk_icol_base[c2] + call_icol_off[c2, w2] + jj // 16
                rows = (jj % 16)[None, :] + 16 * np.arange(8)[:, None]
                idx_core[rows, ic[None, :]] = vals.astype(np.int16)[None, :]
        sched.idx_arrs.append(idx_core)
        sched.dl_arrs.append(dl_core.astype(BF16))
        ap = np.empty((P, nblocks), np.float32)
        ap[:] = a_pad[k * nsp : (k + 1) * nsp].reshape(nblocks, P).T
        sched.a_packed.append(ap)
    return sched


def build_nc(s: Schedule, repeats: int = 1, skip: frozenset = frozenset()):
    import concourse.bacc as bacc
    import concourse.mybir as mybir
    import concourse.tile as tile
    from concourse import library_config

    f32 = mybir.dt.float32
    bf16 = mybir.dt.bfloat16
    i16 = mybir.dt.int16
    AF = mybir.ActivationFunctionType
    OP = mybir.AluOpType

    nc = bacc.Bacc("TRN2", target_bir_lowering=False, debug=False,
                   num_devices=N_CORES, num_swdge_queues=NQUEUES)

    x1 = nc.dram_tensor("x1", [s.npad, D], bf16, kind="ExternalInput")
    xown0 = nc.dram_tensor("xown0", [s.nsp, D], bf16, kind="ExternalInput")
    idx_all = nc.dram_tensor("idx_all", [P, s.total_icols], i16,
                             kind="ExternalInput")
    dl_all = nc.dram_tensor("dl_all", [P, s.total_tiles], bf16,
                            kind="ExternalInput")
    a_pk = nc.dram_tensor("a_pk", [P, s.nblocks], f32, kind="ExternalInput")
    w_in = [nc.dram_tensor(f"w{i}", [D, D], bf16, kind="ExternalInput")
            for i in range(3)]
    brep_in = [nc.dram_tensor(f"brep{i}", [P, D], f32, kind="ExternalInput")
               for i in range(3)]
    wr_in = nc.dram_tensor("wr", [D, O], bf16, kind="ExternalInput")
    brr_in = nc.dram_tensor("brr", [P, O], f32, kind="ExternalInput")
    iota_in = nc.dram_tensor("iota", [P, P], bf16, kind="ExternalInput")
    ident_in = nc.dram_tensor("ident", [P, P], bf16, kind="ExternalInput")
    out = nc.dram_tensor("out", [s.nsp, O], f32, kind="ExternalOutput")

    shard = [nc.dram_tensor(f"shard{i}", [s.nsp, D], bf16, kind="Internal")
             for i in range(2)]
    xfull = [nc.dram_tensor(f"xfull{i}", [s.npad, D], bf16, kind="Internal",
                            addr_space="Shared") for i in range(2)]

    max_tiles = int(s.chunk_tiles.max())
    max_icols = int(s.chunk_icols.max())

    with tile.TileContext(nc) as tc, ExitStack() as ctx:
        nc.gpsimd.load_library(library_config.mlp)
        cp = ctx.enter_context(tc.tile_pool(name="consts", bufs=1))
        msgp = ctx.enter_context(tc.tile_pool(name="msg", bufs=2))
        idxp = ctx.enter_context(tc.tile_pool(name="idx", bufs=2))
        dlp = ctx.enter_context(tc.tile_pool(name="dl", bufs=2))
        selp = ctx.enter_context(tc.tile_pool(name="sel", bufs=4))
        xop = ctx.enter_context(tc.tile_pool(name="xo", bufs=3))
        gp = ctx.enter_context(tc.tile_pool(name="g", bufs=3))
        vp = ctx.enter_context(tc.tile_pool(name="v", bufs=3))
        smp = ctx.enter_context(tc.tile_pool(name="sm", bufs=3))
        pgp = ctx.enter_context(tc.tile_pool(name="pg", bufs=2, space="PSUM"))
        p2p = ctx.enter_context(tc.tile_pool(name="p2", bufs=2, space="PSUM"))
        p3p = ctx.enter_context(tc.tile_pool(name="p3", bufs=2, space="PSUM"))
        p4p = ctx.enter_context(tc.tile_pool(name="p4", bufs=2, space="PSUM"))

        w_t, brep_t = [], []
        for i in range(3):
            t = cp.tile([D, D], bf16, tag=f"w{i}")
            nc.sync.dma_start(out=t[:], in_=w_in[i].ap()[:])
            w_t.append(t)
            t = cp.tile([P, D], f32, tag=f"brep{i}")
            nc.sync.dma_start(out=t[:], in_=brep_in[i].ap()[:])
            brep_t.append(t)
        wr_t = cp.tile([D, O], bf16, tag="wr")
        nc.sync.dma_start(out=wr_t[:], in_=wr_in.ap()[:])
        brr_t = cp.tile([P, O], f32, tag="brr")
        nc.sync.dma_start(out=brr_t[:], in_=brr_in.ap()[:])
        iota_t = cp.tile([P, P], bf16, tag="iota")
        nc.sync.dma_start(out=iota_t[:], in_=iota_in.ap()[:])
        ident_t = cp.tile([P, P], bf16, tag="ident")
        nc.sync.dma_start(out=ident_t[:], in_=ident_in.ap()[:])
        apk_t = cp.tile([P, s.nblocks], f32, tag="apk")
        nc.sync.dma_start(out=apk_t[:], in_=a_pk.ap()[:])

        if "localtab" in skip:
            tables = [x1, x1, x1]
        else:
            tables = [x1, xfull[0], xfull[1]]
        xowns = [xown0, shard[0], shard[1]]

        for _rep in range(repeats):
            _build_pass(nc, tc, s, tables, xowns, shard, xfull, out,
                        w_t, brep_t, wr_t, brr_t, iota_t, apk_t, ident_t,
                        msgp, idxp, dlp, selp, xop, gp, vp, smp,
                        pgp, p2p, p3p, p4p,
                        idx_all, dl_all, max_tiles, max_icols, skip)
    nc.compile()
    return nc


def _build_pass(nc, tc, s, tables, xowns, shard, xfull, out,
                w_t, brep_t, wr_t, brr_t, iota_t, apk_t, ident_t,
                msgp, idxp, dlp, selp, xop, gp, vp, smp,
                pgp, p2p, p3p, p4p,
                idx_all, dl_all, max_tiles, max_icols, skip=frozenset()):
    import concourse.mybir as mybir

    f32 = mybir.dt.float32
    bf16 = mybir.dt.bfloat16
    i16 = mybir.dt.int16
    AF = mybir.ActivationFunctionType
    OP = mybir.AluOpType
    if True:
        for layer in range(3):
            table_ap = tables[layer].ap()
            xown_ap = xowns[layer].ap()
            for c, bl in enumerate(s.chunks):
                tiles_c = int(s.chunk_tiles[c])
                icols_c = int(s.chunk_icols[c])
                msg_t = msgp.tile([P, max_tiles, D], bf16, tag="msg")
                idx_t = idxp.tile([P, max_icols], i16, tag="idx")
                dl_t = dlp.tile([P, max_tiles], bf16, tag="dl")
                ic0 = int(s.chunk_icol_base[c])
                tb0 = int(s.chunk_tile_base[c])
                nc.sync.dma_start(out=idx_t[:, :icols_c],
                                  in_=idx_all.ap()[:, ic0 : ic0 + icols_c])
                nc.sync.dma_start(out=dl_t[:, :tiles_c],
                                  in_=dl_all.ap()[:, tb0 : tb0 + tiles_c])
                if "gather" in skip:
                    nc.vector.memset(msg_t[:, 0, :], 0.0)
                for w in range(NW):
                    if "gather" in skip:
                        continue
                    ntw = int(s.tiles_cw[c, w])
                    if ntw == 0:
                        continue
                    to = int(s.call_tile_off[c, w])
                    io = int(s.call_icol_off[c, w])
                    assert ntw * P <= 8192, "dma_gather call too large for HW"
                    nc.gpsimd.dma_gather(
                        msg_t[:, to : to + ntw, :],
                        table_ap[WSIZE * w :, :],
                        idx_t[:, io : io + ntw * 8],
                        ntw * P,
                        ntw * P,
                        D,
                        single_packet=False,
                        queue_num=w % NQUEUES,
                    )
                if "compute" in skip:
                    for b in bl:
                        if layer < 2:
                            nc.sync.dma_start(
                                out=shard[layer].ap()[b * P : (b + 1) * P, :],
                                in_=msg_t[:, int(s.tile_pos[b, 0]), :])
                    continue
                for b in bl:
                    psum_g = pgp.tile([P, P], f32, tag="pg")
                    first = True
                    for w in range(NW):
                        for t in range(int(s.tw[b, w])):
                            ctl = int(s.tile_pos[b, w]) + t
                            if "sel" in skip:
                                sel = iota_t
                            else:
                                sel = selp.tile([P, P], bf16, tag="sel")
                                nc.vector.tensor_tensor(
                                    out=sel[:],
                                    in0=dl_t[:, ctl : ctl + 1].to_broadcast([P, P]),
                                    in1=iota_t[:],
                                    op=OP.is_equal,
                                )
                            if "aggmm" not in skip:
                                nc.tensor.matmul(
                                    out=psum_g[:], lhsT=msg_t[:, ctl, :],
                                    rhs=sel[:], start=first, stop=False,
                                )
                                first = False
                    xo = xop.tile([P, D], bf16, tag="xo")
                    nc.sync.dma_start(out=xo[:],
                                      in_=xown_ap[b * P : (b + 1) * P, :])
                    nc.tensor.matmul(out=psum_g[:], lhsT=xo[:], rhs=ident_t[:],
                                     start=first, stop=True)
                    g_sb = gp.tile([P, P], bf16, tag="g")
                    nc.vector.tensor_copy(out=g_sb[:], in_=psum_g[:])
                    psum2 = p2p.tile([P, D], f32, tag="p2")
                    nc.tensor.matmul(out=psum2[:], lhsT=g_sb[:],
                                     rhs=w_t[layer][:], start=True, stop=True)
                    acol = apk_t[:, b : b + 1]
                    v = vp.tile([P, D], f32, tag="v")
                    nc.vector.tensor_scalar(
                        out=v[:], in0=psum2[:], scalar1=acol, scalar2=None,
                        op0=OP.mult,
                    )
                    wv = vp.tile([P, D], f32, tag="wv")
                    nc.vector.tensor_tensor(out=wv[:], in0=v[:],
                                            in1=brep_t[layer][:], op=OP.add)
                    if layer < 2:
                        xn = smp.tile([P, D], bf16, tag="xn")
                        nc.scalar.activation(xn[:], wv[:], AF.Relu, scale=acol)
                        nc.sync.dma_start(
                            out=shard[layer].ap()[b * P : (b + 1) * P, :],
                            in_=xn[:])
                    else:
                        o3 = smp.tile([P, D], bf16, tag="o3")
                        nc.scalar.activation(o3[:], wv[:], AF.Relu)
                        psum3 = p3p.tile([P, P], bf16, tag="p3")
                        nc.tensor.transpose(out=psum3[:], in_=o3[:],
                                            identity=ident_t[:])
                        tt = gp.tile([P, P], bf16, tag="tt")
                        nc.vector.tensor_copy(out=tt[:], in_=psum3[:])
                        psum4 = p4p.tile([P, O], f32, tag="p4")
                        nc.tensor.matmul(out=psum4[:], lhsT=tt[:], rhs=wr_t[:],
                                         start=True, stop=True)
                        zr = smp.tile([P, O], f32, tag="zr")
                        nc.vector.tensor_tensor(out=zr[:], in0=psum4[:],
                                                in1=brr_t[:], op=OP.add)
                        sg = smp.tile([P, O], f32, tag="sg")
                        nc.scalar.activation(sg[:], zr[:], AF.Sigmoid)
                        ro = smp.tile([P, O], f32, tag="ro")
                        nc.scalar.activation(ro[:], sg[:], AF.Copy,
                                             scale=0.8, bias=0.1)
                        nc.sync.dma_start(
                            out=out.ap()[b * P : (b + 1) * P, :], in_=ro[:])
            if layer < 2 and "ag" not in skip:
                nc.gpsimd.collective_compute(
                    "AllGather",
                    mybir.AluOpType.bypass,
                    replica_groups=[list(range(N_CORES))],
                    ins=[shard[layer].ap()[:]],
                    outs=[xfull[layer].ap()[:]],
                )


def build_inmaps(s: Schedule, x: np.ndarray, W0, b0, W1, b1, W2, b2, Wr, br):
    x_pad = np.zeros((s.npad, D), np.float32)
    nodes = np.arange(s.n, dtype=np.int64)
    pid_map = (nodes // s.ns0) * s.nsp + nodes % s.ns0
    x_pad[pid_map] = x
    x1 = (x_pad * s.a_pad[:, None]).astype(BF16)

    consts = {
        "x1": x1,
        "w0": np.asarray(W0, BF16), "w1": np.asarray(W1, BF16),
        "w2": np.asarray(W2, BF16),
        "brep0": np.tile(np.asarray(b0, np.float32), (P, 1)),
        "brep1": np.tile(np.asarray(b1, np.float32), (P, 1)),
        "brep2": np.tile(np.asarray(b2, np.float32), (P, 1)),
        "wr": np.asarray(Wr, BF16),
        "brr": np.tile(np.asarray(br, np.float32), (P, 1)),
        "iota": np.tile(np.arange(P, dtype=BF16), (P, 1)),
        "ident": np.eye(P, dtype=BF16),
    }
    in_maps = []
    for k in range(N_CORES):
        m = dict(consts)
        m["xown0"] = np.ascontiguousarray(x1[k * s.nsp : (k + 1) * s.nsp])
        m["idx_all"] = s.idx_arrs[k]
        m["dl_all"] = s.dl_arrs[k]
        m["a_pk"] = s.a_packed[k]
        in_maps.append(m)
    return in_maps


def assemble_output(s: Schedule, results: list) -> np.ndarray:
    out = np.empty((s.n, O), np.float32)
    for k in range(N_CORES):
        lo = k * s.ns0
        hi = min((k + 1) * s.ns0, s.n)
        out[lo:hi] = results[k]["out"][: hi - lo]
    return out


def run(x, edge_index, W0, b0, W1, b1, W2, b2, Wr, br, n, ns0, **run_kwargs):
    from concourse.bass_utils import run_bass_kernel_spmd

    s = build_schedule(np.asarray(edge_index), n, ns0)
    nc = build_nc(s)
    in_maps = build_inmaps(s, np.asarray(x, np.float32), W0, b0, W1, b1, W2,
                           b2, Wr, br)
    res = run_bass_kernel_spmd(nc, in_maps, core_ids=list(range(N_CORES)),
                               **run_kwargs)
    return assemble_output(s, res.results), res


def kernel(x, edge_index, W0, b0, W1, b1, W2, b2, Wr, br):
    out, _ = run(x, edge_index, W0, b0, W1, b1, W2, b2, Wr, br,
                 n=100000, ns0=12500)
    return out



# revision 14
# speedup vs baseline: 2.4877x; 2.3090x over previous
"""GCN (3x GCNConv + readout) on 8 Trainium2 NeuronCores.

Strategy (graph/data parallel over destination nodes):
  - Node rows are sharded across 8 cores by destination; each core owns its
    node shard and all edges pointing into it. Weights are replicated.
  - Math reformulation: with a = deg^-0.5 and x' = a*x (prescaled rows),
        layer(x) = relu(a*( (A0 @ x' + x'_self) @ W ) + b)
    where A0 is the *unweighted* 0/1 adjacency. The per-edge norm
    a[src]*a[dst] factorizes away entirely.
  - Per 128-row destination block, the aggregation A0 @ x' is computed as a
    sum of PE matmuls accumulated in PSUM: for each 128-edge tile,
        psum[d, r] += msg_tile[e, d]^T @ sel_tile[e, r]
    where msg_tile rows are gathered source rows (dma_gather, 512 B/row) and
    sel_tile[e, r] = (dst_local[e] == r) is built on DVE via is_equal against
    an IOTA constant. The self term is one extra matmul against identity.
    Output rows are produced in order -> plain sequential DMA writes.
  - dma_gather indices are int16; the node table is addressed through 4
    fixed 32768-row windows with window-relative indices.
  - Between layers, 6.4 MB node shards are AllGather'd so every core has the
    full prescaled table for the next layer's gathers.
"""

import math
from contextlib import ExitStack
from dataclasses import dataclass, field

import ml_dtypes
import numpy as np

BF16 = ml_dtypes.bfloat16

P = 128
NW = 4            # gather windows
NQUEUES = 4       # SWDGE queues (gather descgen core pairs)
WSIZE = 32768     # int16 window
CH = 6            # blocks per chunk (gather granularity)
D = 128           # feature dim
O = 16            # readout dim
N_CORES = 8


@dataclass
class Schedule:
    n: int                     # real node count
    ns0: int                   # real nodes per core
    nsp: int                   # padded nodes per core (mult of 128)
    npad: int                  # padded total nodes
    nblocks: int               # blocks per core
    chunks: list               # list of block-lists
    tw: np.ndarray             # [nblocks, NW] tiles per (block, window)
    tile_pos: np.ndarray       # [nblocks, NW] chunk-local tile offset
    call_tile_off: np.ndarray  # [nchunks, NW]
    call_icol_off: np.ndarray  # [nchunks, NW]
    tiles_cw: np.ndarray       # [nchunks, NW]
    chunk_tiles: np.ndarray    # [nchunks]
    chunk_icols: np.ndarray    # [nchunks]
    chunk_tile_base: np.ndarray
    chunk_icol_base: np.ndarray
    total_tiles: int
    total_icols: int
    # per-core data
    idx_arrs: list = field(default_factory=list)   # [128, total_icols] int16
    dl_arrs: list = field(default_factory=list)    # [128, total_tiles] f32
    a_packed: list = field(default_factory=list)   # [128, nblocks] f32
    a_pad: np.ndarray | None = None                # [npad] f32


def build_schedule(edge_index: np.ndarray, n: int, ns0: int) -> Schedule:
    src, dst = edge_index[0].astype(np.int64), edge_index[1].astype(np.int64)
    e = src.shape[0]
    nsp = ((ns0 + P - 1) // P) * P
    npad = N_CORES * nsp
    nblocks = nsp // P
    assert npad <= NW * WSIZE

    deg = (1.0 + np.bincount(dst, minlength=n)).astype(np.float32)
    a = deg ** np.float32(-0.5)
    a_pad = np.ones(npad, np.float32)
    nodes = np.arange(n, dtype=np.int64)
    pid_map = (nodes // ns0) * nsp + nodes % ns0
    a_pad[pid_map] = a

    src_pid = (src // ns0) * nsp + src % ns0
    k_arr = dst // ns0
    dst_loc = dst % ns0
    b_arr = dst_loc // P
    dl_arr = (dst_loc % P).astype(np.float32)
    w_arr = src_pid // WSIZE
    idx16 = (src_pid - w_arr * WSIZE).astype(np.int16)

    ngroups = N_CORES * nblocks * NW
    key = (k_arr * nblocks + b_arr) * NW + w_arr
    cnt = np.bincount(key, minlength=ngroups).reshape(N_CORES, nblocks, NW)
    tw = np.ceil(cnt.max(axis=0) / P).astype(np.int64)          # [nblocks, NW]

    chunks = [list(range(c, min(c + CH, nblocks))) for c in range(0, nblocks, CH)]
    nchunks = len(chunks)

    tile_pos = np.zeros((nblocks, NW), np.int64)
    call_tile_off = np.zeros((nchunks, NW), np.int64)
    call_icol_off = np.zeros((nchunks, NW), np.int64)
    tiles_cw = np.zeros((nchunks, NW), np.int64)
    chunk_tiles = np.zeros(nchunks, np.int64)
    chunk_icols = np.zeros(nchunks, np.int64)
    for c, bl in enumerate(chunks):
        off = 0
        ioff = 0
        for w in range(NW):
            call_tile_off[c, w] = off
            call_icol_off[c, w] = ioff
            for b in bl:
                tile_pos[b, w] = off
                off += tw[b, w]
            tiles_cw[c, w] = off - call_tile_off[c, w]
            ioff += tiles_cw[c, w] * 8          # 128 slots/tile -> 8 int16 cols
        chunk_tiles[c] = off
        chunk_icols[c] = ioff
    chunk_tile_base = np.concatenate([[0], np.cumsum(chunk_tiles)[:-1]])
    chunk_icol_base = np.concatenate([[0], np.cumsum(chunk_icols)[:-1]])
    total_tiles = int(chunk_tiles.sum())
    total_icols = int(chunk_icols.sum())

    sched = Schedule(
        n=n, ns0=ns0, nsp=nsp, npad=npad, nblocks=nblocks, chunks=chunks,
        tw=tw, tile_pos=tile_pos, call_tile_off=call_tile_off,
        call_icol_off=call_icol_off, tiles_cw=tiles_cw,
        chunk_tiles=chunk_tiles, chunk_icols=chunk_icols,
        chunk_tile_base=chunk_tile_base, chunk_icol_base=chunk_icol_base,
        total_tiles=total_tiles, total_icols=total_icols, a_pad=a_pad,
    )

    # per-edge slot assignment (vectorized), per core
    order = np.argsort(key, kind="stable")
    grp_start = np.zeros(ngroups + 1, np.int64)
    np.cumsum(cnt.reshape(-1), out=grp_start[1:])
    rank = np.arange(e, dtype=np.int64) - grp_start[key[order]]

    ch_of_b = np.array([b // CH for b in range(nblocks)], np.int64)
    total_slots = total_tiles * P
    for k in range(N_CORES):
        sel = k_arr[order] == k
        eo = order[sel]
        r = rank[sel]
        b = b_arr[eo]
        w = w_arr[eo]
        c = ch_of_b[b]
        stl = tile_pos[b, w]                      # chunk-local seg tile off
        part = r % P
        ctl = stl + r // P                        # chunk-local tile
        gtl = sched.chunk_tile_base[c] + ctl      # global tile
        # dl array
        dl_core = np.full((P, total_tiles), -1.0, np.float32)
        dl_core[part, gtl] = dl_arr[eo]
        # slot values in global slot order (tile*128 + partition)
        slot_vals = np.full(total_slots, -1, np.int32)
        slot_vals[gtl * P + part] = idx16[eo].astype(np.int32)
        # Pad slots must hold valid in-window indices (interior -1 is
        # illegal), but a constant pad value creates an HBM hotspot that
        # slows the whole gather ~1.7x. Cycle each call's own real indices
        # into its pad slots instead.
        idx_core = np.zeros((P, total_icols), np.int16)
        for c2 in range(len(chunks)):
            for w2 in range(NW):
                ntw = int(tiles_cw[c2, w2])
                if ntw == 0:
                    continue
                s0 = (sched.chunk_tile_base[c2] + call_tile_off[c2, w2]) * P
                vals = slot_vals[s0 : s0 + ntw * P].copy()
                pad = vals < 0
                real = vals[~pad]
                npd = int(pad.sum())
                if npd:
                    if len(real):
                        vals[pad] = real[np.arange(npd) % len(real)]
                    else:
                        vals[pad] = 0
                jj = np.arange(ntw * P)
                ic = sched.chunk_icol_base[c2] + call_icol_off[c2, w2] + jj // 16
                rows = (jj % 16)[None, :] + 16 * np.arange(8)[:, None]
                idx_core[rows, ic[None, :]] = vals.astype(np.int16)[None, :]
        sched.idx_arrs.append(idx_core)
        sched.dl_arrs.append(dl_core.astype(BF16))
        ap = np.empty((P, nblocks), np.float32)
        ap[:] = a_pad[k * nsp : (k + 1) * nsp].reshape(nblocks, P).T
        sched.a_packed.append(ap)
    return sched


def build_nc(s: Schedule, repeats: int = 1, skip: frozenset = frozenset()):
    import concourse.bacc as bacc
    import concourse.mybir as mybir
    import concourse.tile as tile
    from concourse import library_config

    f32 = mybir.dt.float32
    bf16 = mybir.dt.bfloat16
    i16 = mybir.dt.int16
    AF = mybir.ActivationFunctionType
    OP = mybir.AluOpType

    nc = bacc.Bacc("TRN2", target_bir_lowering=False, debug=False,
                   num_devices=N_CORES, num_swdge_queues=NQUEUES)

    x1 = nc.dram_tensor("x1", [s.npad, D], bf16, kind="ExternalInput")
    xown0 = nc.dram_tensor("xown0", [s.nsp, D], bf16, kind="ExternalInput")
    idx_all = nc.dram_tensor("idx_all", [P, s.total_icols], i16,
                             kind="ExternalInput")
    dl_all = nc.dram_tensor("dl_all", [P, s.total_tiles], bf16,
                            kind="ExternalInput")
    a_pk = nc.dram_tensor("a_pk", [P, s.nblocks], f32, kind="ExternalInput")
    w_in = [nc.dram_tensor(f"w{i}", [D, D], bf16, kind="ExternalInput")
            for i in range(3)]
    brep_in = [nc.dram_tensor(f"brep{i}", [P, D], f32, kind="ExternalInput")
               for i in range(3)]
    wr_in = nc.dram_tensor("wr", [D, O], bf16, kind="ExternalInput")
    brr_in = nc.dram_tensor("brr", [P, O], f32, kind="ExternalInput")
    iota_in = nc.dram_tensor("iota", [P, P], bf16, kind="ExternalInput")
    ident_in = nc.dram_tensor("ident", [P, P], bf16, kind="ExternalInput")
    out = nc.dram_tensor("out", [s.nsp, O], f32, kind="ExternalOutput")

    shard = [nc.dram_tensor(f"shard{i}", [s.nsp, D], bf16, kind="Internal")
             for i in range(2)]
    xfull = [nc.dram_tensor(f"xfull{i}", [s.npad, D], bf16, kind="Internal",
                            addr_space="Shared") for i in range(2)]

    max_tiles = int(s.chunk_tiles.max())
    max_icols = int(s.chunk_icols.max())

    with tile.TileContext(nc) as tc, ExitStack() as ctx:
        nc.gpsimd.load_library(library_config.mlp)
        cp = ctx.enter_context(tc.tile_pool(name="consts", bufs=1))
        msgp = ctx.enter_context(tc.tile_pool(name="msg", bufs=2))
        idxp = ctx.enter_context(tc.tile_pool(name="idx", bufs=2))
        dlp = ctx.enter_context(tc.tile_pool(name="dl", bufs=2))
        selp = ctx.enter_context(tc.tile_pool(name="sel", bufs=4))
        xop = ctx.enter_context(tc.tile_pool(name="xo", bufs=3))
        gp = ctx.enter_context(tc.tile_pool(name="g", bufs=3))
        vp = ctx.enter_context(tc.tile_pool(name="v", bufs=3))
        smp = ctx.enter_context(tc.tile_pool(name="sm", bufs=3))
        pgp = ctx.enter_context(tc.tile_pool(name="pg", bufs=2, space="PSUM"))
        p2p = ctx.enter_context(tc.tile_pool(name="p2", bufs=2, space="PSUM"))
        p3p = ctx.enter_context(tc.tile_pool(name="p3", bufs=2, space="PSUM"))
        p4p = ctx.enter_context(tc.tile_pool(name="p4", bufs=2, space="PSUM"))

        w_t, brep_t = [], []
        for i in range(3):
            t = cp.tile([D, D], bf16, tag=f"w{i}")
            nc.sync.dma_start(out=t[:], in_=w_in[i].ap()[:])
            w_t.append(t)
            t = cp.tile([P, D], f32, tag=f"brep{i}")
            nc.sync.dma_start(out=t[:], in_=brep_in[i].ap()[:])
            brep_t.append(t)
        wr_t = cp.tile([D, O], bf16, tag="wr")
        nc.sync.dma_start(out=wr_t[:], in_=wr_in.ap()[:])
        brr_t = cp.tile([P, O], f32, tag="brr")
        nc.sync.dma_start(out=brr_t[:], in_=brr_in.ap()[:])
        iota_t = cp.tile([P, P], bf16, tag="iota")
        nc.sync.dma_start(out=iota_t[:], in_=iota_in.ap()[:])
        ident_t = cp.tile([P, P], bf16, tag="ident")
        nc.sync.dma_start(out=ident_t[:], in_=ident_in.ap()[:])
        apk_t = cp.tile([P, s.nblocks], f32, tag="apk")
        nc.sync.dma_start(out=apk_t[:], in_=a_pk.ap()[:])

        if "localtab" in skip:
            tables = [x1, x1, x1]
        else:
            tables = [x1, xfull[0], xfull[1]]
        xowns = [xown0, shard[0], shard[1]]

        for _rep in range(repeats):
            _build_pass(nc, tc, s, tables, xowns, shard, xfull, out,
                        w_t, brep_t, wr_t, brr_t, iota_t, apk_t, ident_t,
                        msgp, idxp, dlp, selp, xop, gp, vp, smp,
                        pgp, p2p, p3p, p4p,
                        idx_all, dl_all, max_tiles, max_icols, skip)
    nc.compile()
    return nc


def _build_pass(nc, tc, s, tables, xowns, shard, xfull, out,
                w_t, brep_t, wr_t, brr_t, iota_t, apk_t, ident_t,
                msgp, idxp, dlp, selp, xop, gp, vp, smp,
                pgp, p2p, p3p, p4p,
                idx_all, dl_all, max_tiles, max_icols, skip=frozenset()):
    import concourse.mybir as mybir

    f32 = mybir.dt.float32
    bf16 = mybir.dt.bfloat16
    i16 = mybir.dt.int16
    AF = mybir.ActivationFunctionType
    OP = mybir.AluOpType
    if True:
        for layer in range(3):
            table_ap = tables[layer].ap()
            xown_ap = xowns[layer].ap()
            for c, bl in enumerate(s.chunks):
                tiles_c = int(s.chunk_tiles[c])
                icols_c = int(s.chunk_icols[c])
                msg_t = msgp.tile([P, max_tiles, D], bf16, tag="msg")
                idx_t = idxp.tile([P, max_icols], i16, tag="idx")
                dl_t = dlp.tile([P, max_tiles], bf16, tag="dl")
                ic0 = int(s.chunk_icol_base[c])
                tb0 = int(s.chunk_tile_base[c])
                nc.sync.dma_start(out=idx_t[:, :icols_c],
                                  in_=idx_all.ap()[:, ic0 : ic0 + icols_c])
                nc.sync.dma_start(out=dl_t[:, :tiles_c],
                                  in_=dl_all.ap()[:, tb0 : tb0 + tiles_c])
                if "gather" in skip:
                    nc.vector.memset(msg_t[:, 0, :], 0.0)
                for w in range(NW):
                    if "gather" in skip:
                        continue
                    ntw = int(s.tiles_cw[c, w])
                    if ntw == 0:
                        continue
                    to = int(s.call_tile_off[c, w])
                    io = int(s.call_icol_off[c, w])
                    assert ntw * P <= 8192, "dma_gather call too large for HW"
                    nc.gpsimd.dma_gather(
                        msg_t[:, to : to + ntw, :],
                        table_ap[WSIZE * w :, :],
                        idx_t[:, io : io + ntw * 8],
                        ntw * P,
                        ntw * P,
                        D,
                        single_packet=False,
                        queue_num=w % NQUEUES,
                    )
                if "compute" in skip:
                    for b in bl:
                        if layer < 2:
                            nc.sync.dma_start(
                                out=shard[layer].ap()[b * P : (b + 1) * P, :],
                                in_=msg_t[:, int(s.tile_pos[b, 0]), :])
                    continue
                for b in bl:
                    psum_g = pgp.tile([P, P], f32, tag="pg")
                    first = True
                    for w in range(NW):
                        for t in range(int(s.tw[b, w])):
                            ctl = int(s.tile_pos[b, w]) + t
                            if "sel" in skip:
                                sel = iota_t
                            else:
                                sel = selp.tile([P, P], bf16, tag="sel")
                                nc.vector.tensor_tensor(
                                    out=sel[:],
                                    in0=dl_t[:, ctl : ctl + 1].to_broadcast([P, P]),
                                    in1=iota_t[:],
                                    op=OP.is_equal,
                                )
                            if "aggmm" not in skip:
                                nc.tensor.matmul(
                                    out=psum_g[:], lhsT=msg_t[:, ctl, :],
                                    rhs=sel[:], start=first, stop=False,
                                )
                                first = False
                    xo = xop.tile([P, D], bf16, tag="xo")
                    nc.sync.dma_start(out=xo[:],
                                      in_=xown_ap[b * P : (b + 1) * P, :])
                    nc.tensor.matmul(out=psum_g[:], lhsT=xo[:], rhs=ident_t[:],
                                     start=first, stop=True)
                    g_sb = gp.tile([P, P], bf16, tag="g")
                    nc.vector.tensor_copy(out=g_sb[:], in_=psum_g[:])
                    psum2 = p2p.tile([P, D], f32, tag="p2")
                    nc.tensor.matmul(out=psum2[:], lhsT=g_sb[:],
                                     rhs=w_t[layer][:], start=True, stop=True)
                    acol = apk_t[:, b : b + 1]
                    v = vp.tile([P, D], f32, tag="v")
                    nc.vector.tensor_scalar(
                        out=v[:], in0=psum2[:], scalar1=acol, scalar2=None,
                        op0=OP.mult,
                    )
                    wv = vp.tile([P, D], f32, tag="wv")
                    nc.vector.tensor_tensor(out=wv[:], in0=v[:],
                                            in1=brep_t[layer][:], op=OP.add)
                    if layer < 2:
                        xn = smp.tile([P, D], bf16, tag="xn")
                        nc.scalar.activation(xn[:], wv[:], AF.Relu, scale=acol)
                        nc.sync.dma_start(
                            out=shard[layer].ap()[b * P : (b + 1) * P, :],
                            in_=xn[:])
                    else:
                        o3 = smp.tile([P, D], bf16, tag="o3")
                        nc.scalar.activation(o3[:], wv[:], AF.Relu)
                        psum3 = p3p.tile([P, P], bf16, tag="p3")
                        nc.tensor.transpose(out=psum3[:], in_=o3[:],
                                            identity=ident_t[:])
                        tt = gp.tile([P, P], bf16, tag="tt")
                        nc.vector.tensor_copy(out=tt[:], in_=psum3[:])
                        psum4 = p4p.tile([P, O], f32, tag="p4")
                        nc.tensor.matmul(out=psum4[:], lhsT=tt[:], rhs=wr_t[:],
                                         start=True, stop=True)
                        zr = smp.tile([P, O], f32, tag="zr")
                        nc.vector.tensor_tensor(out=zr[:], in0=psum4[:],
                                                in1=brr_t[:], op=OP.add)
                        sg = smp.tile([P, O], f32, tag="sg")
                        nc.scalar.activation(sg[:], zr[:], AF.Sigmoid)
                        ro = smp.tile([P, O], f32, tag="ro")
                        nc.scalar.activation(ro[:], sg[:], AF.Copy,
                                             scale=0.8, bias=0.1)
                        nc.sync.dma_start(
                            out=out.ap()[b * P : (b + 1) * P, :], in_=ro[:])
            if layer < 2 and "ag" not in skip:
                nc.gpsimd.collective_compute(
                    "AllGather",
                    mybir.AluOpType.bypass,
                    replica_groups=[list(range(N_CORES))],
                    ins=[shard[layer].ap()[:]],
                    outs=[xfull[layer].ap()[:]],
                )


def build_inmaps(s: Schedule, x: np.ndarray, W0, b0, W1, b1, W2, b2, Wr, br):
    x_pad = np.zeros((s.npad, D), np.float32)
    nodes = np.arange(s.n, dtype=np.int64)
    pid_map = (nodes // s.ns0) * s.nsp + nodes % s.ns0
    x_pad[pid_map] = x
    x1 = (x_pad * s.a_pad[:, None]).astype(BF16)

    consts = {
        "x1": x1,
        "w0": np.asarray(W0, BF16), "w1": np.asarray(W1, BF16),
        "w2": np.asarray(W2, BF16),
        "brep0": np.tile(np.asarray(b0, np.float32), (P, 1)),
        "brep1": np.tile(np.asarray(b1, np.float32), (P, 1)),
        "brep2": np.tile(np.asarray(b2, np.float32), (P, 1)),
        "wr": np.asarray(Wr, BF16),
        "brr": np.tile(np.asarray(br, np.float32), (P, 1)),
        "iota": np.tile(np.arange(P, dtype=BF16), (P, 1)),
        "ident": np.eye(P, dtype=BF16),
    }
    in_maps = []
    for k in range(N_CORES):
        m = dict(consts)
        m["xown0"] = np.ascontiguousarray(x1[k * s.nsp : (k + 1) * s.nsp])
        m["idx_all"] = s.idx_arrs[k]
        m["dl_all"] = s.dl_arrs[k]
        m["a_pk"] = s.a_packed[k]
        in_maps.append(m)
    return in_maps


def assemble_output(s: Schedule, results: list) -> np.ndarray:
    out = np.empty((s.n, O), np.float32)
    for k in range(N_CORES):
        lo = k * s.ns0
        hi = min((k + 1) * s.ns0, s.n)
        out[lo:hi] = results[k]["out"][: hi - lo]
    return out


def run(x, edge_index, W0, b0, W1, b1, W2, b2, Wr, br, n, ns0, **run_kwargs):
    from concourse.bass_utils import run_bass_kernel_spmd

    s = build_schedule(np.asarray(edge_index), n, ns0)
    nc = build_nc(s)
    in_maps = build_inmaps(s, np.asarray(x, np.float32), W0, b0, W1, b1, W2,
                           b2, Wr, br)
    res = run_bass_kernel_spmd(nc, in_maps, core_ids=list(range(N_CORES)),
                               **run_kwargs)
    return assemble_output(s, res.results), res


def kernel(x, edge_index, W0, b0, W1, b1, W2, b2, Wr, br):
    out, _ = run(x, edge_index, W0, b0, W1, b1, W2, b2, Wr, br,
                 n=100000, ns0=12500)
    return out



# revision 29
# speedup vs baseline: 2.9641x; 1.1915x over previous
"""GCN (3x GCNConv + readout) on 8 Trainium2 NeuronCores.

Strategy (graph/data parallel over destination nodes):
  - Node rows are sharded across 8 cores by destination; each core owns its
    node shard and all edges pointing into it. Weights are replicated.
  - Math reformulation: with a = deg^-0.5 and x' = a*x (prescaled rows),
        layer(x) = relu(a*( (A0 @ x' + x'_self) @ W ) + b)
    where A0 is the *unweighted* 0/1 adjacency. The per-edge norm
    a[src]*a[dst] factorizes away entirely.
  - Per 128-row destination block, the aggregation A0 @ x' is computed as a
    sum of PE matmuls accumulated in PSUM: for each 128-edge tile,
        psum[d, r] += msg_tile[e, d]^T @ sel_tile[e, r]
    where msg_tile rows are gathered source rows (dma_gather, 512 B/row) and
    sel_tile[e, r] = (dst_local[e] == r) is built on DVE via is_equal against
    an IOTA constant. The self term is one extra matmul against identity.
    Output rows are produced in order -> plain sequential DMA writes.
  - dma_gather indices are int16; the node table is addressed through 4
    fixed 32768-row windows with window-relative indices.
  - Between layers, 6.4 MB node shards are AllGather'd so every core has the
    full prescaled table for the next layer's gathers.
"""

import math
from contextlib import ExitStack
from dataclasses import dataclass, field

import ml_dtypes
import numpy as np

BF16 = ml_dtypes.bfloat16

import os

P = 128
NW = 4            # gather windows
# SWDGE queues (gather descgen core pairs); overridable for the watchdog
# fallback path in kernel().
NQUEUES = int(os.environ.get("GCN_NQUEUES", "4"))
WSIZE = 32768     # int16 window reach (max offset range)
WBASE = 25088     # window base stride: 4 overlapping windows, balanced load
CH = 6            # blocks per chunk (gather granularity)
D = 128           # feature dim
O = 16            # readout dim
N_CORES = 8


@dataclass
class Schedule:
    n: int                     # real node count
    ns0: int                   # real nodes per core
    nsp: int                   # padded nodes per core (mult of 128)
    npad: int                  # padded total nodes
    nblocks: int               # blocks per core
    chunks: list               # list of block-lists
    tw: np.ndarray             # [nblocks, NW] tiles per (block, window)
    tile_pos: np.ndarray       # [nblocks, NW] chunk-local tile offset
    call_tile_off: np.ndarray  # [nchunks, NW]
    call_icol_off: np.ndarray  # [nchunks, NW]
    tiles_cw: np.ndarray       # [nchunks, NW]
    chunk_tiles: np.ndarray    # [nchunks]
    chunk_icols: np.ndarray    # [nchunks]
    chunk_tile_base: np.ndarray
    chunk_icol_base: np.ndarray
    total_tiles: int
    total_icols: int
    # per-core data
    idx_arrs: list = field(default_factory=list)   # [128, total_icols] int16
    dl_arrs: list = field(default_factory=list)    # [128, total_tiles] bf16
    a_packed: list = field(default_factory=list)   # [128, nblocks] f32
    a2_packed: list = field(default_factory=list)  # [128, nblocks] f32 (a^2)
    inva_rows: list = field(default_factory=list)  # [1, nsp] bf16 (1/a)
    a_pad: np.ndarray | None = None                # [npad] f32


def build_schedule(edge_index: np.ndarray, n: int, ns0: int) -> Schedule:
    src, dst = edge_index[0].astype(np.int64), edge_index[1].astype(np.int64)
    e = src.shape[0]
    nsp = ((ns0 + P - 1) // P) * P
    npad = N_CORES * nsp
    nblocks = nsp // P
    assert npad <= NW * WBASE
    assert WBASE + WSIZE >= 2 * WBASE  # window reach covers its bucket

    deg = (1.0 + np.bincount(dst, minlength=n)).astype(np.float32)
    a = deg ** np.float32(-0.5)
    a_pad = np.ones(npad, np.float32)
    nodes = np.arange(n, dtype=np.int64)
    pid_map = (nodes // ns0) * nsp + nodes % ns0
    a_pad[pid_map] = a

    src_pid = (src // ns0) * nsp + src % ns0
    k_arr = dst // ns0
    dst_loc = dst % ns0
    b_arr = dst_loc // P
    dl_arr = (dst_loc % P).astype(np.float32)
    w_arr = src_pid // WBASE
    idx16 = (src_pid - w_arr * WBASE).astype(np.int16)

    ngroups = N_CORES * nblocks * NW
    key = (k_arr * nblocks + b_arr) * NW + w_arr
    cnt = np.bincount(key, minlength=ngroups).reshape(N_CORES, nblocks, NW)
    tw = np.ceil(cnt.max(axis=0) / P).astype(np.int64)          # [nblocks, NW]

    chunks = [list(range(c, min(c + CH, nblocks))) for c in range(0, nblocks, CH)]
    nchunks = len(chunks)

    tile_pos = np.zeros((nblocks, NW), np.int64)
    call_tile_off = np.zeros((nchunks, NW), np.int64)
    call_icol_off = np.zeros((nchunks, NW), np.int64)
    tiles_cw = np.zeros((nchunks, NW), np.int64)
    chunk_tiles = np.zeros(nchunks, np.int64)
    chunk_icols = np.zeros(nchunks, np.int64)
    for c, bl in enumerate(chunks):
        off = 0
        ioff = 0
        for w in range(NW):
            call_tile_off[c, w] = off
            call_icol_off[c, w] = ioff
            for b in bl:
                tile_pos[b, w] = off
                off += tw[b, w]
            tiles_cw[c, w] = off - call_tile_off[c, w]
            ioff += tiles_cw[c, w] * 8          # 128 slots/tile -> 8 int16 cols
        chunk_tiles[c] = off
        chunk_icols[c] = ioff
    chunk_tile_base = np.concatenate([[0], np.cumsum(chunk_tiles)[:-1]])
    chunk_icol_base = np.concatenate([[0], np.cumsum(chunk_icols)[:-1]])
    total_tiles = int(chunk_tiles.sum())
    total_icols = int(chunk_icols.sum())

    sched = Schedule(
        n=n, ns0=ns0, nsp=nsp, npad=npad, nblocks=nblocks, chunks=chunks,
        tw=tw, tile_pos=tile_pos, call_tile_off=call_tile_off,
        call_icol_off=call_icol_off, tiles_cw=tiles_cw,
        chunk_tiles=chunk_tiles, chunk_icols=chunk_icols,
        chunk_tile_base=chunk_tile_base, chunk_icol_base=chunk_icol_base,
        total_tiles=total_tiles, total_icols=total_icols, a_pad=a_pad,
    )

    # per-edge slot assignment (vectorized), per core
    order = np.argsort(key, kind="stable")
    grp_start = np.zeros(ngroups + 1, np.int64)
    np.cumsum(cnt.reshape(-1), out=grp_start[1:])
    rank = np.arange(e, dtype=np.int64) - grp_start[key[order]]

    ch_of_b = np.array([b // CH for b in range(nblocks)], np.int64)
    total_slots = total_tiles * P
    for k in range(N_CORES):
        sel = k_arr[order] == k
        eo = order[sel]
        r = rank[sel]
        b = b_arr[eo]
        w = w_arr[eo]
        c = ch_of_b[b]
        stl = tile_pos[b, w]                      # chunk-local seg tile off
        part = r % P
        ctl = stl + r // P                        # chunk-local tile
        gtl = sched.chunk_tile_base[c] + ctl      # global tile
        # dl array
        dl_core = np.full((P, total_tiles), -1.0, np.float32)
        dl_core[part, gtl] = dl_arr[eo]
        # slot values in global slot order (tile*128 + partition)
        slot_vals = np.full(total_slots, -1, np.int32)
        slot_vals[gtl * P + part] = idx16[eo].astype(np.int32)
        # Pad slots must hold valid in-window indices (interior -1 is
        # illegal), but a constant pad value creates an HBM hotspot that
        # slows the whole gather ~1.7x. Cycle each call's own real indices
        # into its pad slots instead.
        idx_core = np.zeros((P, total_icols), np.int16)
        for c2 in range(len(chunks)):
            for w2 in range(NW):
                ntw = int(tiles_cw[c2, w2])
                if ntw == 0:
                    continue
                s0 = (sched.chunk_tile_base[c2] + call_tile_off[c2, w2]) * P
                vals = slot_vals[s0 : s0 + ntw * P].copy()
                pad = vals < 0
                real = vals[~pad]
                npd = int(pad.sum())
                if npd:
                    if len(real):
                        vals[pad] = real[np.arange(npd) % len(real)]
                    else:
                        vals[pad] = 0
                jj = np.arange(ntw * P)
                ic = sched.chunk_icol_base[c2] + call_icol_off[c2, w2] + jj // 16
                rows = (jj % 16)[None, :] + 16 * np.arange(8)[:, None]
                idx_core[rows, ic[None, :]] = vals.astype(np.int16)[None, :]
        sched.idx_arrs.append(idx_core)
        sched.dl_arrs.append(dl_core.astype(BF16))
        ap = np.empty((P, nblocks), np.float32)
        ap[:] = a_pad[k * nsp : (k + 1) * nsp].reshape(nblocks, P).T
        sched.a_packed.append(ap)
        sched.a2_packed.append(ap * ap)
        sched.inva_rows.append(
            (1.0 / a_pad[k * nsp : (k + 1) * nsp])[None, :].astype(BF16))
    return sched


def build_nc(s: Schedule, repeats: int = 1, skip: frozenset = frozenset()):
    import concourse.bacc as bacc
    import concourse.mybir as mybir
    import concourse.tile as tile
    from concourse import library_config

    f32 = mybir.dt.float32
    bf16 = mybir.dt.bfloat16
    i16 = mybir.dt.int16
    AF = mybir.ActivationFunctionType
    OP = mybir.AluOpType

    nc = bacc.Bacc("TRN2", target_bir_lowering=False, debug=False,
                   num_devices=N_CORES, num_swdge_queues=NQUEUES)

    x1 = nc.dram_tensor("x1", [s.npad, D], bf16, kind="ExternalInput")
    xown0 = nc.dram_tensor("xown0", [s.nsp, D], bf16, kind="ExternalInput")
    idx_all = nc.dram_tensor("idx_all", [P, s.total_icols], i16,
                             kind="ExternalInput")
    dl_all = nc.dram_tensor("dl_all", [P, s.total_tiles], bf16,
                            kind="ExternalInput")
    a_pk = nc.dram_tensor("a_pk", [P, s.nblocks], f32, kind="ExternalInput")
    a2_pk = nc.dram_tensor("a2_pk", [P, s.nblocks], f32, kind="ExternalInput")
    inva_in = nc.dram_tensor("inva", [1, s.nsp], bf16, kind="ExternalInput")
    w_in = [nc.dram_tensor(f"w{i}", [D, D], bf16, kind="ExternalInput")
            for i in range(3)]
    brow_in = [nc.dram_tensor(f"brow{i}", [1, D], bf16, kind="ExternalInput")
               for i in range(3)]
    wr_in = nc.dram_tensor("wr", [D, O], bf16, kind="ExternalInput")
    brr_in = nc.dram_tensor("brr", [P, O], f32, kind="ExternalInput")
    iota_in = nc.dram_tensor("iota", [P, P], bf16, kind="ExternalInput")
    ident_in = nc.dram_tensor("ident", [P, P], bf16, kind="ExternalInput")
    out = nc.dram_tensor("out", [s.nsp, O], f32, kind="ExternalOutput")

    shard = [nc.dram_tensor(f"shard{i}", [s.nsp, D], bf16, kind="Internal")
             for i in range(2)]
    xfull = [nc.dram_tensor(f"xfull{i}", [s.npad, D], bf16, kind="Internal",
                            addr_space="Shared") for i in range(2)]

    max_tiles = int(s.chunk_tiles.max())
    max_icols = int(s.chunk_icols.max())

    with tile.TileContext(nc) as tc, ExitStack() as ctx:
        nc.gpsimd.load_library(library_config.mlp)
        cp = ctx.enter_context(tc.tile_pool(name="consts", bufs=1))
        msgp = ctx.enter_context(tc.tile_pool(name="msg", bufs=2))
        idxp = ctx.enter_context(tc.tile_pool(name="idx", bufs=2))
        dlp = ctx.enter_context(tc.tile_pool(name="dl", bufs=2))
        selp = ctx.enter_context(tc.tile_pool(name="sel", bufs=4))
        xop = ctx.enter_context(tc.tile_pool(name="xo", bufs=3))
        gp = ctx.enter_context(tc.tile_pool(name="g", bufs=3))
        vp = ctx.enter_context(tc.tile_pool(name="v", bufs=3))
        smp = ctx.enter_context(tc.tile_pool(name="sm", bufs=3))
        pgp = ctx.enter_context(tc.tile_pool(name="pg", bufs=2, space="PSUM"))
        p2p = ctx.enter_context(tc.tile_pool(name="p2", bufs=2, space="PSUM"))
        p3p = ctx.enter_context(tc.tile_pool(name="p3", bufs=2, space="PSUM"))
        p4p = ctx.enter_context(tc.tile_pool(name="p4", bufs=2, space="PSUM"))

        w_t, brow_t = [], []
        for i in range(3):
            t = cp.tile([D, D], bf16, tag=f"w{i}")
            nc.sync.dma_start(out=t[:], in_=w_in[i].ap()[:])
            w_t.append(t)
            t = cp.tile([1, D], bf16, tag=f"brow{i}")
            nc.sync.dma_start(out=t[:], in_=brow_in[i].ap()[:])
            brow_t.append(t)
        wr_t = cp.tile([D, O], bf16, tag="wr")
        nc.sync.dma_start(out=wr_t[:], in_=wr_in.ap()[:])
        brr_t = cp.tile([P, O], f32, tag="brr")
        nc.sync.dma_start(out=brr_t[:], in_=brr_in.ap()[:])
        iota_t = cp.tile([P, P], bf16, tag="iota")
        nc.sync.dma_start(out=iota_t[:], in_=iota_in.ap()[:])
        ident_t = cp.tile([P, P], bf16, tag="ident")
        nc.sync.dma_start(out=ident_t[:], in_=ident_in.ap()[:])
        apk_t = cp.tile([P, s.nblocks], f32, tag="apk")
        nc.sync.dma_start(out=apk_t[:], in_=a_pk.ap()[:])
        apk2_t = cp.tile([P, s.nblocks], f32, tag="apk2")
        nc.sync.dma_start(out=apk2_t[:], in_=a2_pk.ap()[:])
        inva_t = cp.tile([1, s.nsp], bf16, tag="inva")
        nc.sync.dma_start(out=inva_t[:], in_=inva_in.ap()[:])

        if "localtab" in skip:
            tables = [x1, x1, x1]
        else:
            tables = [x1, xfull[0], xfull[1]]
        xowns = [xown0, shard[0], shard[1]]

        for _rep in range(repeats):
            _build_pass(nc, tc, s, tables, xowns, shard, xfull, out,
                        w_t, brow_t, wr_t, brr_t, iota_t, apk_t, apk2_t,
                        inva_t, ident_t,
                        msgp, idxp, dlp, selp, xop, gp, vp, smp,
                        pgp, p2p, p3p, p4p,
                        idx_all, dl_all, max_tiles, max_icols, skip)
    nc.compile()
    return nc


def _build_pass(nc, tc, s, tables, xowns, shard, xfull, out,
                w_t, brow_t, wr_t, brr_t, iota_t, apk_t, apk2_t,
                inva_t, ident_t,
                msgp, idxp, dlp, selp, xop, gp, vp, smp,
                pgp, p2p, p3p, p4p,
                idx_all, dl_all, max_tiles, max_icols, skip=frozenset()):
    import concourse.mybir as mybir

    f32 = mybir.dt.float32
    bf16 = mybir.dt.bfloat16
    i16 = mybir.dt.int16
    AF = mybir.ActivationFunctionType
    OP = mybir.AluOpType
    TWMAX = int(s.tw.max())
    if True:
        for layer in range(3):
            table_ap = tables[layer].ap()
            xown_ap = xowns[layer].ap()
            for c, bl in enumerate(s.chunks):
                tiles_c = int(s.chunk_tiles[c])
                icols_c = int(s.chunk_icols[c])
                msg_t = msgp.tile([P, max_tiles, D], bf16, tag="msg")
                idx_t = idxp.tile([P, max_icols], i16, tag="idx")
                dl_t = dlp.tile([P, max_tiles], bf16, tag="dl")
                ic0 = int(s.chunk_icol_base[c])
                tb0 = int(s.chunk_tile_base[c])
                nc.sync.dma_start(out=idx_t[:, :icols_c],
                                  in_=idx_all.ap()[:, ic0 : ic0 + icols_c])
                nc.sync.dma_start(out=dl_t[:, :tiles_c],
                                  in_=dl_all.ap()[:, tb0 : tb0 + tiles_c])
                if "gather" in skip:
                    nc.vector.memset(msg_t[:, 0, :], 0.0)
                for w in range(NW):
                    if "gather" in skip:
                        continue
                    ntw = int(s.tiles_cw[c, w])
                    if ntw == 0:
                        continue
                    to = int(s.call_tile_off[c, w])
                    io = int(s.call_icol_off[c, w])
                    assert ntw * P <= 8192, "dma_gather call too large for HW"
                    nc.gpsimd.dma_gather(
                        msg_t[:, to : to + ntw, :],
                        table_ap[WBASE * w :, :],
                        idx_t[:, io : io + ntw * 8],
                        ntw * P,
                        ntw * P,
                        D,
                        single_packet=False,
                        queue_num=w % NQUEUES,
                    )
                if "compute" in skip:
                    for b in bl:
                        if layer < 2:
                            nc.sync.dma_start(
                                out=shard[layer].ap()[b * P : (b + 1) * P, :],
                                in_=msg_t[:, int(s.tile_pos[b, 0]), :])
                    continue
                for b in bl:
                    psum_g = pgp.tile([P, P], f32, tag="pg")
                    first = True
                    for w in range(NW):
                        ntw_b = int(s.tw[b, w])
                        if ntw_b == 0:
                            continue
                        pos = int(s.tile_pos[b, w])
                        if "sel" in skip:
                            sel_ap = iota_t[:].unsqueeze(1).broadcast_to(
                                [P, ntw_b, P])
                        else:
                            sel_run = selp.tile([P, TWMAX, P], bf16, tag="sel")
                            nc.vector.tensor_tensor(
                                out=sel_run[:, 0:ntw_b, :],
                                in0=dl_t[:, pos : pos + ntw_b].to_broadcast(
                                    [P, ntw_b, P]),
                                in1=iota_t[:].unsqueeze(1).broadcast_to(
                                    [P, ntw_b, P]),
                                op=OP.is_equal,
                            )
                            sel_ap = sel_run
                        if "aggmm" not in skip:
                            for t in range(ntw_b):
                                nc.tensor.matmul(
                                    out=psum_g[:],
                                    lhsT=msg_t[:, pos + t, :],
                                    rhs=sel_ap[:, t, :],
                                    start=first, stop=False,
                                )
                                first = False
                    xo = xop.tile([P, D], bf16, tag="xo")
                    nc.sync.dma_start(out=xo[:],
                                      in_=xown_ap[b * P : (b + 1) * P, :])
                    nc.tensor.matmul(out=psum_g[:], lhsT=xo[:], rhs=ident_t[:],
                                     start=first, stop=True)
                    g_sb = gp.tile([P, P], bf16, tag="g")
                    nc.vector.tensor_copy(out=g_sb[:], in_=psum_g[:])
                    psum2 = p2p.tile([P, D], f32, tag="p2")
                    nc.tensor.matmul(out=psum2[:],
                                     lhsT=inva_t[0:1, b * P : (b + 1) * P],
                                     rhs=brow_t[layer][:],
                                     start=True, stop=False)
                    nc.tensor.matmul(out=psum2[:], lhsT=g_sb[:],
                                     rhs=w_t[layer][:], start=False, stop=True)
                    acol = apk_t[:, b : b + 1]
                    a2col = apk2_t[:, b : b + 1]
                    if layer < 2:
                        xn = smp.tile([P, D], bf16, tag="xn")
                        nc.scalar.activation(xn[:], psum2[:], AF.Relu,
                                             scale=a2col)
                        nc.sync.dma_start(
                            out=shard[layer].ap()[b * P : (b + 1) * P, :],
                            in_=xn[:])
                    else:
                        o3 = smp.tile([P, D], bf16, tag="o3")
                        nc.scalar.activation(o3[:], psum2[:], AF.Relu,
                                             scale=acol)
                        psum3 = p3p.tile([P, P], bf16, tag="p3")
                        nc.tensor.transpose(out=psum3[:], in_=o3[:],
                                            identity=ident_t[:])
                        tt = gp.tile([P, P], bf16, tag="tt")
                        nc.vector.tensor_copy(out=tt[:], in_=psum3[:])
                        psum4 = p4p.tile([P, O], f32, tag="p4")
                        nc.tensor.matmul(out=psum4[:], lhsT=tt[:], rhs=wr_t[:],
                                         start=True, stop=True)
                        zr = smp.tile([P, O], f32, tag="zr")
                        nc.vector.tensor_tensor(out=zr[:], in0=psum4[:],
                                                in1=brr_t[:], op=OP.add)
                        sg = smp.tile([P, O], f32, tag="sg")
                        nc.scalar.activation(sg[:], zr[:], AF.Sigmoid)
                        ro = smp.tile([P, O], f32, tag="ro")
                        nc.scalar.activation(ro[:], sg[:], AF.Copy,
                                             scale=0.8, bias=0.1)
                        nc.sync.dma_start(
                            out=out.ap()[b * P : (b + 1) * P, :], in_=ro[:])
            if layer < 2 and "ag" not in skip:
                nc.gpsimd.collective_compute(
                    "AllGather",
                    mybir.AluOpType.bypass,
                    replica_groups=[list(range(N_CORES))],
                    ins=[shard[layer].ap()[:]],
                    outs=[xfull[layer].ap()[:]],
                )


def build_inmaps(s: Schedule, x: np.ndarray, W0, b0, W1, b1, W2, b2, Wr, br):
    x_pad = np.zeros((s.npad, D), np.float32)
    nodes = np.arange(s.n, dtype=np.int64)
    pid_map = (nodes // s.ns0) * s.nsp + nodes % s.ns0
    x_pad[pid_map] = x
    x1 = (x_pad * s.a_pad[:, None]).astype(BF16)

    consts = {
        "x1": x1,
        "w0": np.asarray(W0, BF16), "w1": np.asarray(W1, BF16),
        "w2": np.asarray(W2, BF16),
        "brow0": np.asarray(b0, BF16)[None, :],
        "brow1": np.asarray(b1, BF16)[None, :],
        "brow2": np.asarray(b2, BF16)[None, :],
        "wr": np.asarray(Wr, BF16),
        "brr": np.tile(np.asarray(br, np.float32), (P, 1)),
        "iota": np.tile(np.arange(P, dtype=BF16), (P, 1)),
        "ident": np.eye(P, dtype=BF16),
    }
    in_maps = []
    for k in range(N_CORES):
        m = dict(consts)
        m["xown0"] = np.ascontiguousarray(x1[k * s.nsp : (k + 1) * s.nsp])
        m["idx_all"] = s.idx_arrs[k]
        m["dl_all"] = s.dl_arrs[k]
        m["a_pk"] = s.a_packed[k]
        m["a2_pk"] = s.a2_packed[k]
        m["inva"] = s.inva_rows[k]
        in_maps.append(m)
    return in_maps


def assemble_output(s: Schedule, results: list) -> np.ndarray:
    out = np.empty((s.n, O), np.float32)
    for k in range(N_CORES):
        lo = k * s.ns0
        hi = min((k + 1) * s.ns0, s.n)
        out[lo:hi] = results[k]["out"][: hi - lo]
    return out


def run(x, edge_index, W0, b0, W1, b1, W2, b2, Wr, br, n, ns0, **run_kwargs):
    from concourse.bass_utils import run_bass_kernel_spmd

    s = build_schedule(np.asarray(edge_index), n, ns0)
    nc = build_nc(s)
    in_maps = build_inmaps(s, np.asarray(x, np.float32), W0, b0, W1, b1, W2,
                           b2, Wr, br)
    res = run_bass_kernel_spmd(nc, in_maps, core_ids=list(range(N_CORES)),
                               **run_kwargs)
    return assemble_output(s, res.results), res


def _kernel_inproc(x, edge_index, W0, b0, W1, b1, W2, b2, Wr, br):
    out, _ = run(x, edge_index, W0, b0, W1, b1, W2, b2, Wr, br,
                 n=100000, ns0=12500)
    return out


_ARG_NAMES = ["x", "edge_index", "W0", "b0", "W1", "b1", "W2", "b2",
              "Wr", "br"]


def kernel(x, edge_index, W0, b0, W1, b1, W2, b2, Wr, br):
    """Robust entry: run the device pipeline in a watchdog subprocess.

    A rare intermittent device-side hang has been observed with 4 SWDGE
    queues; the subprocess + timeout + retry (final attempt with 1 queue
    and a core reset) makes the call reliable either way.
    """
    import subprocess
    import sys
    import tempfile

    args = dict(zip(_ARG_NAMES, [x, edge_index, W0, b0, W1, b1, W2, b2,
                                 Wr, br]))
    with tempfile.TemporaryDirectory() as td:
        inp = os.path.join(td, "in.npz")
        outp = os.path.join(td, "out.npy")
        np.savez(inp, **{k: np.asarray(v) for k, v in args.items()})
        # (nqueues, timeout_s); first attempt may pay a fresh neuronx-cc
        # compile, so its budget is generous.
        attempts = [(4, 900), (4, 600), (1, 1200)]
        err = None
        for i, (nq, tmo) in enumerate(attempts):
            env = dict(os.environ)
            env["GCN_NQUEUES"] = str(nq)
            if i > 0:
                env["NEURON_RT_RESET_CORES"] = "1"
            try:
                r = subprocess.run(
                    [sys.executable, os.path.abspath(__file__),
                     "--child", inp, outp],
                    timeout=tmo, env=env)
                if r.returncode == 0 and os.path.exists(outp):
                    return np.load(outp)
                err = f"child rc={r.returncode}"
            except subprocess.TimeoutExpired:
                err = f"timeout after {tmo}s (nqueues={nq})"
    raise RuntimeError(f"kernel: all attempts failed ({err})")


def _child_main(inp: str, outp: str) -> None:
    data = np.load(inp)
    out = _kernel_inproc(**{k: data[k] for k in _ARG_NAMES})
    np.save(outp + ".tmp.npy", out)
    os.replace(outp + ".tmp.npy", outp)


if __name__ == "__main__":
    import sys as _sys

    if len(_sys.argv) == 4 and _sys.argv[1] == "--child":
        _child_main(_sys.argv[2], _sys.argv[3])

